# revision 2
# baseline (speedup 1.0000x reference)
"""Trainium2 Bass kernel v2 for nn_MixedLlamaDecoderLayer_732.

Architecture (8 cores), designed to minimize collective bytes:
  - X^T replicated to every core (host, bf16); ln1 stats computed on own
    512-token slice + 16KB AllGather; ln1_w folded into Wq/Wk/Wv on host.
  - QKV + RoPE + causal attention head-sharded (4 Q heads / 1 KV head per
    core) exactly like v1.
  - Single AllToAll (4.2MB/2.1MB) converts head-sharded attention output
    [512 hd, T] to token-sharded [NH*HD, 512own].
  - Everything after is LOCAL per 512-token slice with full weights
    streamed from HBM: o_proj, +residual, ln2 (ln2_w folded into gate/up),
    gate/up/silu, down, +residual. Zero further collectives.
  - Optional fp8(e4m3) DoubleRow matmuls (2x PE) for MLP / o_proj / QKV.
"""

import os
import sys
from contextlib import ExitStack

os.environ.setdefault("JAX_PLATFORMS", "cpu")
if "/opt/trn_rl_repo" not in sys.path:
    sys.path.insert(0, "/opt/trn_rl_repo")

import numpy as np
import ml_dtypes

import concourse.bass as bass
import concourse.bacc as bacc
import concourse.tile as tile
from concourse import mybir

BF16 = mybir.dt.bfloat16
F32 = mybir.dt.float32
FP8 = mybir.dt.float8e4
AF = mybir.ActivationFunctionType
ALU = mybir.AluOpType
DR = mybir.MatmulPerfMode.DoubleRow

NCORES = 8
B, S, HID = 4, 1024, 4096
T = B * S
NH, NKV, HD = 32, 8, 128
FF = 11008
EPS = 1e-6
THETA = 10000.0

TS = T // NCORES               # 512 own tokens
QC = NH * HD // NCORES         # 512 q cols/core (4 heads)
KC = HD                        # 128 kv cols/core
NHT = HID // 128               # 32 hid k-tiles
NTT = T // 512                 # 8 token tiles
NFT = FF // 128                # 86 ff tiles
NOG_O = 8                      # o_proj out col groups (512 each)
NOG_D = 32                     # down out col groups (128 each)
SCALE = 1.0 / float(np.sqrt(HD))

# fp8 config (empirically: fp8 e4m3 fails the 2e-2 gate -- keep all False)
F8_MLP = False
F8_O = False
F8_QKV = False
SW = 64.0      # weight scale
SA = 8.0       # activation scale (h2, xn, ao)
SG = 8.0       # gt scale


def build_nc():
    nc = bacc.Bacc("TRN2", target_bir_lowering=False, debug=False,
                   num_devices=NCORES)
    WQ_DT = FP8 if F8_QKV else BF16
    WO_DT = FP8 if F8_O else BF16
    WM_DT = FP8 if F8_MLP else BF16
    c_o = (1.0 / (SA * SW)) if F8_O else 1.0
    c_qkv = (1.0 / (SA * SW)) if F8_QKV else 1.0
    c_g = (1.0 / (SA * SW)) if F8_MLP else 1.0          # psg -> g
    c_gt = (SG * c_g * c_g) if F8_MLP else 1.0           # psg*psu -> gt*SG
    c_d = (1.0 / (SG * SW)) if F8_MLP else 1.0

    d = {}
    ein = lambda n, s, t: nc.dram_tensor(n, s, t, kind="ExternalInput")
    d["xT_t"] = ein("xT_t", [128, NTT, NHT // 4, 4, 512], BF16)
    d["hidT_t"] = ein("hidT_t", [128, NHT, TS], F32)
    d["wq_t"] = ein("wq_t", [128, NHT, QC], WQ_DT)
    d["wk_t"] = ein("wk_t", [128, NHT, KC], WQ_DT)
    d["wv_t"] = ein("wv_t", [128, NHT, KC], WQ_DT)
    d["wo_t"] = ein("wo_t", [128, NOG_O, NHT, 512], WO_DT)
    d["gate_t"] = ein("gate_t", [128, NFT, NHT, 128], WM_DT)
    d["up_t"] = ein("up_t", [128, NFT, NHT, 128], WM_DT)
    d["down_t"] = ein("down_t", [128, NOG_D, NFT, 128], WM_DT)
    d["cosT"] = ein("cosT", [128, T], BF16)
    d["sinS"] = ein("sinS", [128, T], BF16)
    d["maskT"] = ein("maskT", [128, (S // 128) * S], BF16)
    d["ident"] = ein("ident", [128, 128], BF16)
    d["ones128"] = ein("ones128", [128, 1], BF16)
    d["ones1"] = ein("ones1", [1, 128], BF16)
    out_c = nc.dram_tensor("out_c", [NHT, 128, TS], F32, kind="ExternalOutput")

    AO_DT = FP8 if F8_O else BF16
    ag_st_in = nc.dram_tensor("ag_st_in", [1, TS], F32)
    ag_st = nc.dram_tensor("ag_st", [NCORES, TS], F32, addr_space="Shared")
    a2a_in = nc.dram_tensor("a2a_in", [NH * HD, TS], AO_DT)
    a2a_out = nc.dram_tensor("a2a_out", [NH * HD, TS], AO_DT)
    RG = [list(range(NCORES))]

    with tile.TileContext(nc) as tc:
        with tc.tile_pool(name="consts", bufs=1) as consts:
            ident = consts.tile([128, 128], BF16)
            nc.sync.dma_start(ident[:], d["ident"][:])
            ones128 = consts.tile([128, 1], BF16)
            nc.sync.dma_start(ones128[:], d["ones128"][:])
            ones1 = consts.tile([1, 128], BF16)
            nc.sync.dma_start(ones1[:], d["ones1"][:])
            eps128 = consts.tile([128, 1], F32)
            nc.gpsimd.memset(eps128[:], EPS)

            # ======== Stage 0: ln1 stats on own tokens -> AllGather ========
            with (
                tc.tile_pool(name="s0", bufs=1) as s0,
                tc.tile_pool(name="s0t", bufs=2) as s0t,
                tc.tile_pool(name="s0p", bufs=1, space="PSUM") as s0p,
            ):
                hstat = s0.tile([128, NHT, TS], F32)
                nc.sync.dma_start(hstat[:], d["hidT_t"][:])
                ps_ssq = s0p.tile([1, TS], F32)
                for k in range(NHT):
                    sq = s0t.tile([128, TS], BF16, tag="sq", name=f"sq{k}")
                    nc.scalar.activation(sq[:], hstat[:, k, :], AF.Square)
                    nc.tensor.matmul(ps_ssq[:], ones128[:], sq[:],
                                     start=(k == 0), stop=(k == NHT - 1))
                st = s0t.tile([1, TS], F32, tag="st")
                nc.scalar.activation(st[:], ps_ssq[:], AF.Sqrt,
                                     scale=1.0 / HID, bias=eps128[0:1, 0:1])
                rt = s0t.tile([1, TS], F32, tag="rt")
                nc.vector.reciprocal(rt[:], st[:])
                nc.sync.dma_start(ag_st_in[:], rt[:])
            nc.gpsimd.collective_compute(
                "AllGather", ALU.bypass, replica_groups=RG,
                ins=[ag_st_in[:]], outs=[ag_st[:]])

            # rb_all: rstd broadcast [128, T] bf16 (lives in bc_keep)
            bc_es = ExitStack()
            bc_keep = bc_es.enter_context(tc.tile_pool(name="bc_keep", bufs=1))
            rb_all = bc_keep.tile([128, T], BF16, tag="rb_all")
            with (
                tc.tile_pool(name="rbt", bufs=2) as rbt,
                tc.tile_pool(name="rbp", bufs=2, space="PSUM") as rbp,
            ):
                rstd_all = rbt.tile([1, T], F32, tag="ra")
                nc.sync.dma_start(
                    rstd_all[:],
                    ag_st[:].rearrange("(p a) t -> p (a t)", p=1))
                rstd_bf = rbt.tile([1, T], BF16, tag="rb")
                nc.vector.tensor_copy(rstd_bf[:], rstd_all[:])
                for tt in range(NTT):
                    psr = rbp.tile([128, 512], F32, tag="psr", name=f"psr{tt}")
                    nc.tensor.matmul(psr[:], ones1[:],
                                     rstd_bf[:, tt * 512:(tt + 1) * 512],
                                     start=True, stop=True)
                    nc.vector.tensor_copy(rb_all[:, tt * 512:(tt + 1) * 512],
                                          psr[:])

            # ======== Stage 1: QKV + RoPE (head shard, full tokens) ========
            qT = bc_keep.tile([128, 4 * T], BF16, tag="qT")
            kT = bc_keep.tile([128, T], BF16, tag="kT")
            vS = bc_keep.tile([128, T], BF16, tag="vS")
            cosT = bc_keep.tile([128, T], BF16, tag="cosT")
            nc.sync.dma_start(cosT[:], d["cosT"][:])
            sinS = bc_keep.tile([128, T], BF16, tag="sinS")
            nc.sync.dma_start(sinS[:], d["sinS"][:])

            with (
                tc.tile_pool(name="b_w", bufs=1) as b_w,
                tc.tile_pool(name="b_x", bufs=3) as b_x,
                tc.tile_pool(name="b_tmp", bufs=2) as b_tmp,
                tc.tile_pool(name="b_ps", bufs=1, space="PSUM") as b_ps,
            ):
                wq = b_w.tile([128, NHT, QC], WQ_DT)
                nc.sync.dma_start(wq[:], d["wq_t"][:])
                wk = b_w.tile([128, NHT, KC], WQ_DT)
                nc.sync.dma_start(wk[:], d["wk_t"][:])
                wv = b_w.tile([128, NHT, KC], WQ_DT)
                nc.sync.dma_start(wv[:], d["wv_t"][:])

                def rope(dst, dst_off, ps, cs_off):
                    c_lo = cosT[0:64, cs_off:cs_off + 512]
                    c_hi = cosT[64:128, cs_off:cs_off + 512]
                    s_lo = sinS[0:64, cs_off:cs_off + 512]
                    s_hi = sinS[64:128, cs_off:cs_off + 512]
                    t1 = b_tmp.tile([128, 512], F32, tag="ro1", name="ro1")
                    nc.vector.tensor_mul(t1[0:64, :], ps[64:128, :], s_lo)
                    nc.vector.tensor_mul(t1[64:128, :], ps[0:64, :], s_hi)
                    t2 = b_tmp.tile([128, 512], F32, tag="ro2", name="ro2")
                    nc.vector.tensor_mul(t2[0:64, :], ps[0:64, :], c_lo)
                    nc.vector.tensor_mul(t2[64:128, :], ps[64:128, :], c_hi)
                    nc.vector.tensor_add(dst[:, dst_off:dst_off + 512],
                                         t1[:], t2[:])

                for tt in range(NTT):
                    psq = [b_ps.tile([128, 512], F32, tag=f"psq{i}",
                                     name=f"psq{i}_{tt}") for i in range(4)]
                    psk = b_ps.tile([128, 512], F32, tag="psk", name=f"psk_{tt}")
                    psv = b_ps.tile([128, 512], F32, tag="psv", name=f"psv_{tt}")
                    rbs = rb_all[:, tt * 512:(tt + 1) * 512]
                    for h4 in range(NHT // 4):
                        xt4 = b_x.tile([128, 4, 512], BF16, tag="xt",
                                       name=f"xt_{tt}_{h4}")
                        nc.sync.dma_start(xt4[:], d["xT_t"][:, tt, h4, :, :])
                        xn4 = b_x.tile([128, 4, 512], WQ_DT, tag="xn",
                                       name=f"xn_{tt}_{h4}")
                        for a in range(4):
                            if F8_QKV:
                                nc.vector.scalar_tensor_tensor(
                                    xn4[:, a, :], xt4[:, a, :], SA, rbs,
                                    op0=ALU.mult, op1=ALU.mult)
                            else:
                                nc.vector.tensor_mul(xn4[:, a, :],
                                                     xt4[:, a, :], rbs)
                        if F8_QKV:
                            for a2 in range(2):
                                h = h4 * 4 + a2 * 2
                                xs = xn4[:, a2 * 2:a2 * 2 + 2, :]
                                st_, sp = (h == 0), (h + 2 == NHT)
                                for qc in range(4):
                                    nc.tensor.matmul(
                                        psq[qc][:],
                                        wq[:, h:h + 2, qc * 128:(qc + 1) * 128],
                                        xs, start=st_, stop=sp, perf_mode=DR)
                                nc.tensor.matmul(psk[:], wk[:, h:h + 2, :], xs,
                                                 start=st_, stop=sp,
                                                 perf_mode=DR)
                                nc.tensor.matmul(psv[:], wv[:, h:h + 2, :], xs,
                                                 start=st_, stop=sp,
                                                 perf_mode=DR)
                        else:
                            for a in range(4):
                                h = h4 * 4 + a
                                xs = xn4[:, a, :]
                                st_, sp = (h == 0), (h == NHT - 1)
                                for qc in range(4):
                                    nc.tensor.matmul(
                                        psq[qc][:],
                                        wq[:, h, qc * 128:(qc + 1) * 128],
                                        xs, start=st_, stop=sp)
                                nc.tensor.matmul(psk[:], wk[:, h, :], xs,
                                                 start=st_, stop=sp)
                                nc.tensor.matmul(psv[:], wv[:, h, :], xs,
                                                 start=st_, stop=sp)
                    for qc in range(4):
                        ps = psq[qc]
                        if F8_QKV:
                            nc.vector.tensor_scalar_mul(ps[:], ps[:], c_qkv)
                        rope(qT, qc * T + tt * 512, ps, tt * 512)
                    if F8_QKV:
                        nc.vector.tensor_scalar_mul(psk[:], psk[:], c_qkv)
                    rope(kT, tt * 512, psk, tt * 512)
                    vtmp = b_tmp.tile([128, 512], BF16, tag="vtmp",
                                      name=f"vtmp_{tt}")
                    if F8_QKV:
                        nc.vector.tensor_scalar_mul(vtmp[:], psv[:], c_qkv)
                    else:
                        nc.vector.tensor_copy(vtmp[:], psv[:])
                    for s4 in range(4):
                        pvt = b_ps.tile([128, 128], BF16, tag="tpv",
                                        name=f"tpv_{tt}_{s4}", bufs=2)
                        nc.tensor.transpose(pvt[:], vtmp[:, s4 * 128:(s4 + 1) * 128],
                                            ident[:])
                        nc.vector.tensor_copy(
                            vS[:, (tt * 4 + s4) * 128:(tt * 4 + s4 + 1) * 128],
                            pvt[:])

            # ======== Stage 2: causal attention -> a2a_in ========
            with (
                tc.tile_pool(name="c_pt", bufs=2) as c_pt,
                tc.tile_pool(name="c_keep", bufs=1) as c_keep,
                tc.tile_pool(name="c_tmp", bufs=4) as c_tmp,
                tc.tile_pool(name="c_ps", bufs=2, space="PSUM") as c_ps,
                tc.tile_pool(name="c_psd", bufs=2, space="PSUM") as c_psd,
            ):
                maskT = c_keep.tile([128, (S // 128) * S], BF16, tag="maskT")
                nc.sync.dma_start(maskT[:], d["maskT"][:])
                NKT = S // 128
                for b in range(B):
                    for h in range(4):
                        pt = c_pt.tile([128, NKT * S], BF16, tag="pt",
                                       name=f"pt_{b}_{h}")
                        qoff = h * T + b * S
                        for kt in range(NKT):
                            for q2 in range(2):
                                if kt * 128 >= (q2 + 1) * 512:
                                    continue
                                pss = c_ps.tile([128, 512], F32, tag="pss",
                                                name=f"pss_{b}_{h}_{kt}_{q2}")
                                nc.tensor.matmul(
                                    pss[:],
                                    kT[:, b * S + kt * 128: b * S + (kt + 1) * 128],
                                    qT[:, qoff + q2 * 512: qoff + (q2 + 1) * 512],
                                    start=True, stop=True)
                                po = kt * S + q2 * 512
                                nc.vector.scalar_tensor_tensor(
                                    pt[:, po:po + 512], pss[:], SCALE,
                                    maskT[:, kt * S + q2 * 512: kt * S + (q2 + 1) * 512],
                                    op0=ALU.mult, op1=ALU.add)
                                nc.scalar.activation(pt[:, po:po + 512],
                                                     pt[:, po:po + 512], AF.Exp)
                        for q2 in range(2):
                            nk = min(NKT, (q2 + 1) * 4)
                            psd = c_psd.tile([1, 512], F32, tag="psd",
                                             name=f"psd_{b}_{h}_{q2}")
                            for kt in range(nk):
                                nc.tensor.matmul(
                                    psd[:], ones128[:],
                                    pt[:, kt * S + q2 * 512: kt * S + (q2 + 1) * 512],
                                    start=(kt == 0), stop=(kt == nk - 1))
                            dnr = c_tmp.tile([1, 512], F32, tag="dnr", name="dnr")
                            nc.vector.reciprocal(dnr[:], psd[:])
                            dnb = c_tmp.tile([1, 512], BF16, tag="dnb", name="dnb")
                            nc.vector.tensor_copy(dnb[:], dnr[:])
                            psr = c_psd.tile([128, 512], F32, tag="psr",
                                             name=f"psr_{b}_{h}_{q2}")
                            nc.tensor.matmul(psr[:], ones1[:], dnb[:],
                                             start=True, stop=True)
                            rb = c_tmp.tile([128, 512], BF16, tag="rb", name="rb")
                            nc.vector.tensor_copy(rb[:], psr[:])
                            psa = c_ps.tile([128, 512], F32, tag="psa",
                                            name=f"psa_{b}_{h}_{q2}")
                            for kt in range(nk):
                                nc.tensor.matmul(
                                    psa[:],
                                    vS[:, (b * 8 + kt) * 128:(b * 8 + kt + 1) * 128],
                                    pt[:, kt * S + q2 * 512: kt * S + (q2 + 1) * 512],
                                    start=(kt == 0), stop=(kt == nk - 1))
                            ao = c_tmp.tile([128, 512], AO_DT, tag="ao", name="ao")
                            if F8_O:
                                nc.vector.scalar_tensor_tensor(
                                    ao[:], psa[:], SA, rb[:],
                                    op0=ALU.mult, op1=ALU.mult)
                            else:
                                nc.vector.tensor_mul(ao[:], psa[:], rb[:])
                            nc.sync.dma_start(
                                a2a_in[(b * 2 + q2) * 512 + h * 128:
                                       (b * 2 + q2) * 512 + (h + 1) * 128, :],
                                ao[:])
            bc_es.close()
            nc.gpsimd.collective_compute(
                "AllToAll", ALU.bypass, replica_groups=RG,
                ins=[a2a_in[:]], outs=[a2a_out[:]])

            # ======== Stage 3: o_proj + residual + ln2 (local tokens) ========
            d_es = ExitStack()
            d_keep = d_es.enter_context(tc.tile_pool(name="d_keep", bufs=1))
            h1 = d_keep.tile([128, NHT, TS], BF16, tag="h1")
            with (
                tc.tile_pool(name="d_at", bufs=1) as d_at,
                tc.tile_pool(name="d_w", bufs=2) as d_w,
                tc.tile_pool(name="d_r", bufs=2) as d_r,
                tc.tile_pool(name="d_ps", bufs=2, space="PSUM") as d_ps,
            ):
                at_all = d_at.tile([128, NHT, TS], AO_DT)
                nc.sync.dma_start(
                    at_all[:], a2a_out[:].rearrange("(a p) t -> p a t", p=128))
                for og in range(NOG_O):
                    wo_ch = d_w.tile([128, NHT, 512], WO_DT, tag="wo",
                                     name=f"wo_{og}")
                    nc.sync.dma_start(wo_ch[:], d["wo_t"][:, og, :, :])
                    hres = d_r.tile([128, 4, TS], F32, tag="hres",
                                    name=f"hres_{og}")
                    nc.sync.dma_start(hres[:],
                                      d["hidT_t"][:, og * 4:(og + 1) * 4, :])
                    pso = [d_ps.tile([128, TS], F32, tag=f"pso{i}",
                                     name=f"pso{i}_{og}") for i in range(4)]
                    if F8_O:
                        for k2 in range(NHT // 2):
                            st_, sp = (k2 == 0), (k2 == NHT // 2 - 1)
                            ats = at_all[:, k2 * 2:k2 * 2 + 2, :]
                            for oc in range(4):
                                nc.tensor.matmul(
                                    pso[oc][:],
                                    wo_ch[:, k2 * 2:k2 * 2 + 2,
                                          oc * 128:(oc + 1) * 128],
                                    ats, start=st_, stop=sp, perf_mode=DR)
                    else:
                        for k in range(NHT):
                            st_, sp = (k == 0), (k == NHT - 1)
                            ats = at_all[:, k, :]
                            for oc in range(4):
                                nc.tensor.matmul(
                                    pso[oc][:],
                                    wo_ch[:, k, oc * 128:(oc + 1) * 128],
                                    ats, start=st_, stop=sp)
                    for oc in range(4):
                        kk = og * 4 + oc
                        nc.vector.scalar_tensor_tensor(
                            h1[:, kk, :], pso[oc][:], c_o, hres[:, oc, :],
                            op0=ALU.mult, op1=ALU.add)
            # gt_all reserved before h2 so h2's pool can be released
            # (freeing 32KB/partition) before the down-proj weight chunks.
            e_es = ExitStack()
            e_gt = e_es.enter_context(tc.tile_pool(name="e_gt", bufs=1))
            gt_all = e_gt.tile([128, NFT, TS], FP8 if F8_MLP else BF16)
            h2_es = ExitStack()
            p_h2 = h2_es.enter_context(tc.tile_pool(name="p_h2", bufs=1))
            h2 = p_h2.tile([128, NHT, TS], FP8 if F8_MLP else BF16, tag="h2")
            with (
                tc.tile_pool(name="d2_tmp", bufs=2) as d_tmp,
                tc.tile_pool(name="d2_ps", bufs=1, space="PSUM") as d2_ps,
            ):
                ps_ssq2 = d2_ps.tile([1, TS], F32, tag="pssq2")
                for k in range(NHT):
                    sq2 = d_tmp.tile([128, TS], BF16, tag="sq2", name="sq2")
                    nc.scalar.activation(sq2[:], h1[:, k, :], AF.Square)
                    nc.tensor.matmul(ps_ssq2[:], ones128[:], sq2[:],
                                     start=(k == 0), stop=(k == NHT - 1))
                st2 = d_tmp.tile([1, TS], F32, tag="st2")
                nc.scalar.activation(st2[:], ps_ssq2[:], AF.Sqrt,
                                     scale=1.0 / HID, bias=eps128[0:1, 0:1])
                rt2 = d_tmp.tile([1, TS], F32, tag="rt2")
                nc.vector.reciprocal(rt2[:], st2[:])
                rt2b = d_tmp.tile([1, TS], BF16, tag="rt2b")
                nc.vector.tensor_copy(rt2b[:], rt2[:])
                psb = d2_ps.tile([128, TS], F32, tag="psb")
                nc.tensor.matmul(psb[:], ones1[:], rt2b[:], start=True, stop=True)
                rb2 = d_keep.tile([128, TS], BF16, tag="rb2")
                nc.vector.tensor_copy(rb2[:], psb[:])
                for k in range(NHT):
                    nc.vector.scalar_tensor_tensor(
                        h2[:, k, :], h1[:, k, :], SA if F8_MLP else 1.0,
                        rb2[:], op0=ALU.mult, op1=ALU.mult)

            # ======== Stage 4a: gate/up + silu ========
            with (
                tc.tile_pool(name="e_wgu", bufs=2) as e_wgu,
                tc.tile_pool(name="e_tmp", bufs=2) as e_tmp,
                tc.tile_pool(name="e_ps", bufs=2, space="PSUM") as e_ps,
            ):
                for f in range(NFT):
                    g_ch = e_wgu.tile([128, NHT, 128], WM_DT, tag="gch",
                                      name=f"gch_{f}")
                    nc.sync.dma_start(g_ch[:], d["gate_t"][:, f, :, :])
                    u_ch = e_wgu.tile([128, NHT, 128], WM_DT, tag="uch",
                                      name=f"uch_{f}")
                    nc.sync.dma_start(u_ch[:], d["up_t"][:, f, :, :])
                    psg = e_ps.tile([128, TS], F32, tag="psg", name=f"psg_{f}")
                    psu = e_ps.tile([128, TS], F32, tag="psu", name=f"psu_{f}")
                    if F8_MLP:
                        for k2 in range(NHT // 2):
                            st_, sp = (k2 == 0), (k2 == NHT // 2 - 1)
                            h2s = h2[:, k2 * 2:k2 * 2 + 2, :]
                            nc.tensor.matmul(psg[:], g_ch[:, k2 * 2:k2 * 2 + 2, :],
                                             h2s, start=st_, stop=sp,
                                             perf_mode=DR)
                            nc.tensor.matmul(psu[:], u_ch[:, k2 * 2:k2 * 2 + 2, :],
                                             h2s, start=st_, stop=sp,
                                             perf_mode=DR)
                    else:
                        for k in range(NHT):
                            st_, sp = (k == 0), (k == NHT - 1)
                            nc.tensor.matmul(psg[:], g_ch[:, k, :], h2[:, k, :],
                                             start=st_, stop=sp)
                            nc.tensor.matmul(psu[:], u_ch[:, k, :], h2[:, k, :],
                                             start=st_, stop=sp)
                    gsig = e_tmp.tile([128, TS], BF16, tag="gsig", name="gsig")
                    nc.scalar.activation(gsig[:], psg[:], AF.Sigmoid,
                                         scale=c_g)
                    t1 = e_tmp.tile([128, TS], BF16, tag="t1", name="t1")
                    nc.vector.scalar_tensor_tensor(
                        t1[:], psg[:], c_gt, gsig[:],
                        op0=ALU.mult, op1=ALU.mult)
                    nc.vector.tensor_mul(gt_all[:, f, :], t1[:], psu[:])
            h2_es.close()

            # ======== Stage 4b: down + residual ========
            with (
                tc.tile_pool(name="e_wd", bufs=2) as e_wd,
                tc.tile_pool(name="e2_tmp", bufs=2) as e2_tmp,
                tc.tile_pool(name="e2_ps", bufs=2, space="PSUM") as e2_ps,
            ):
                for og in range(NOG_D):
                    d_ch = e_wd.tile([128, NFT, 128], WM_DT, tag="dch",
                                     name=f"dch_{og}")
                    nc.sync.dma_start(d_ch[:], d["down_t"][:, og, :, :])
                    psd = e2_ps.tile([128, TS], F32, tag="psd",
                                     name=f"psd_{og}")
                    if F8_MLP:
                        for f2 in range(NFT // 2):
                            st_, sp = (f2 == 0), (f2 == NFT // 2 - 1)
                            nc.tensor.matmul(
                                psd[:], d_ch[:, f2 * 2:f2 * 2 + 2, :],
                                gt_all[:, f2 * 2:f2 * 2 + 2, :],
                                start=st_, stop=sp, perf_mode=DR)
                    else:
                        for f in range(NFT):
                            st_, sp = (f == 0), (f == NFT - 1)
                            nc.tensor.matmul(
                                psd[:], d_ch[:, f, :], gt_all[:, f, :],
                                start=st_, stop=sp)
                    ot = e2_tmp.tile([128, TS], F32, tag="ot", name="ot")
                    nc.vector.scalar_tensor_tensor(
                        ot[:], psd[:], c_d, h1[:, og, :],
                        op0=ALU.mult, op1=ALU.add)
                    nc.sync.dma_start(out_c[og, :, :], ot[:])
            e_es.close()
            d_es.close()

    nc.compile()
    return nc


def host_prep(inputs):
    bf = ml_dtypes.bfloat16
    f8 = ml_dtypes.float8_e4m3
    hs = np.asarray(inputs["hidden_states"], np.float32)
    pos = np.asarray(inputs["position_ids"]).astype(np.int64).reshape(-1)
    mask = np.asarray(inputs["attn_mask"], np.float32).reshape(S, S)
    Wq = np.asarray(inputs["Wq"], np.float32)
    Wk = np.asarray(inputs["Wk"], np.float32)
    Wv = np.asarray(inputs["Wv"], np.float32)
    Wo = np.asarray(inputs["Wo"], np.float32)
    ln1 = np.asarray(inputs["ln1_w"], np.float32)
    ln2 = np.asarray(inputs["ln2_w"], np.float32)
    wg = np.asarray(inputs["w_gate"], np.float32)
    wu = np.asarray(inputs["w_up"], np.float32)
    wd = np.asarray(inputs["w_down"], np.float32)

    def q8(w):
        return np.asarray(np.clip(w * SW, -240, 240), f8)

    def wcast_q(w):
        return q8(w) if F8_QKV else np.asarray(w, bf)

    def wcast_o(w):
        return q8(w) if F8_O else np.asarray(w, bf)

    def wcast_m(w):
        return q8(w) if F8_MLP else np.asarray(w, bf)

    # fold ln weights into the contracting rows of the next matmul
    Wq_f, Wk_f, Wv_f = Wq * ln1[:, None], Wk * ln1[:, None], Wv * ln1[:, None]
    wg_f, wu_f = wg * ln2[:, None], wu * ln2[:, None]

    hsT = np.ascontiguousarray(hs.T)                       # [HID, T]
    xT_bf = hsT.astype(bf)
    # [128, NTT, 8, 4, 512]
    xT_t = np.ascontiguousarray(
        xT_bf.reshape(NHT, 128, NTT, 512).transpose(1, 2, 0, 3)
        .reshape(128, NTT, NHT // 4, 4, 512))

    invf = 1.0 / (THETA ** (np.arange(0, HD, 2, dtype=np.float32) / HD))
    ang = pos[None, :].astype(np.float32) * np.concatenate([invf, invf])[:, None]
    cosT = np.cos(ang)
    sinS = np.sin(ang)
    sinS[:64] *= -1.0

    maskT = np.ascontiguousarray(mask.T)
    maskT_b = maskT.reshape(S // 128, 128, S).transpose(1, 0, 2).reshape(128, -1)

    def tile_k(w, cols):
        # [HID, cols] -> [128, NHT, cols]
        return np.ascontiguousarray(w.reshape(NHT, 128, cols).transpose(1, 0, 2))

    # full Wo: [NH*HD, HID] -> [128, NOG_O, NHT, 512]
    wo_t = np.ascontiguousarray(
        Wo.reshape(NHT, 128, NOG_O, 512).transpose(1, 2, 0, 3))
    # full gate/up: [HID, FF] -> [128, NFT, NHT, 128]
    gate_t = np.ascontiguousarray(
        wg_f.reshape(NHT, 128, NFT, 128).transpose(1, 2, 0, 3))
    up_t = np.ascontiguousarray(
        wu_f.reshape(NHT, 128, NFT, 128).transpose(1, 2, 0, 3))
    # full down: [FF, HID] -> [128, NOG_D, NFT, 256]
    down_t = np.ascontiguousarray(
        wd.reshape(NFT, 128, NOG_D, 128).transpose(1, 2, 0, 3))

    ident = np.eye(128, dtype=np.float32).astype(bf)
    ones128 = np.ones((128, 1), np.float32).astype(bf)
    ones1 = np.ones((1, 128), np.float32).astype(bf)

    wo_c = wcast_o(wo_t)
    gate_c = wcast_m(gate_t)
    up_c = wcast_m(up_t)
    down_c = wcast_m(down_t)
    cosb = cosT.astype(bf)
    sinb = sinS.astype(bf)
    maskb = maskT_b.astype(bf)

    in_maps = []
    for c in range(NCORES):
        qs, ks = c * QC, c * KC
        hidT_c = np.ascontiguousarray(hsT[:, c * TS:(c + 1) * TS])
        m = {
            "xT_t": xT_t,
            "hidT_t": np.ascontiguousarray(
                hidT_c.reshape(NHT, 128, TS).transpose(1, 0, 2)),
            "wq_t": wcast_q(tile_k(np.ascontiguousarray(Wq_f[:, qs:qs + QC]), QC)),
            "wk_t": wcast_q(tile_k(np.ascontiguousarray(Wk_f[:, ks:ks + KC]), KC)),
            "wv_t": wcast_q(tile_k(np.ascontiguousarray(Wv_f[:, ks:ks + KC]), KC)),
            "wo_t": wo_c,
            "gate_t": gate_c,
            "up_t": up_c,
            "down_t": down_c,
            "cosT": cosb,
            "sinS": sinb,
            "maskT": maskb,
            "ident": ident,
            "ones128": ones128,
            "ones1": ones1,
        }
        in_maps.append(m)
    return in_maps


_NC_CACHE = {}


def get_nc():
    if "nc" not in _NC_CACHE:
        _NC_CACHE["nc"] = build_nc()
    return _NC_CACHE["nc"]


def assemble(results):
    # out_c per core: [NHT, 128, TS] -> [HID, TS]; tokens c*TS..; out [T, HID]
    out = np.empty((T, HID), np.float32)
    for c in range(NCORES):
        oc = results[c]["out_c"].reshape(HID, TS)
        out[c * TS:(c + 1) * TS, :] = oc.T
    return out


def _get_runner():
    if "runner" in _NC_CACHE:
        return _NC_CACHE["runner"]
    import jax
    from jax.sharding import Mesh, PartitionSpec, NamedSharding
    from jax.experimental.shard_map import shard_map
    from concourse import bass2jax, mybir as mb
    from concourse.bass2jax import _bass_exec_p, install_neuronx_cc_hook

    nc = get_nc()
    install_neuronx_cc_hook()
    in_names, out_names, out_avals, zero_outs = [], [], [], []
    partition_name = (nc.partition_id_tensor.name
                      if nc.partition_id_tensor else None)
    for alloc in nc.m.functions[0].allocations:
        if not isinstance(alloc, mb.MemoryLocationSet):
            continue
        name = alloc.memorylocations[0].name
        if alloc.kind == "ExternalInput":
            if name != partition_name:
                in_names.append(name)
        elif alloc.kind == "ExternalOutput":
            out_names.append(name)
            shape = tuple(alloc.tensor_shape)
            dtype = mb.dt.np(alloc.dtype)
            out_avals.append(jax.core.ShapedArray(shape, dtype))
            zero_outs.append(np.zeros(shape, dtype))
    n_params = len(in_names)
    n_outs = len(out_avals)
    all_in_names = list(in_names) + list(out_names)
    if partition_name is not None:
        all_in_names.append(partition_name)

    def _body(*args):
        operands = list(args)
        if partition_name is not None:
            operands.append(bass2jax.partition_id_tensor())
        outs = _bass_exec_p.bind(
            *operands,
            out_avals=tuple(out_avals),
            in_names=tuple(all_in_names),
            out_names=tuple(out_names),
            lowering_input_output_aliases=(),
            sim_require_finite=True,
            sim_require_nnan=True,
            nc=nc,
        )
        return tuple(outs)

    devices = jax.devices()[:NCORES]
    mesh = Mesh(np.asarray(devices), ("core",))
    donate = tuple(range(n_params, n_params + n_outs))
    sharded = jax.jit(
        shard_map(_body, mesh=mesh,
                  in_specs=(PartitionSpec("core"),) * (n_params + n_outs),
                  out_specs=(PartitionSpec("core"),) * n_outs,
                  check_rep=False),
        donate_argnums=donate, keep_unused=True)
    runner = {
        "jax": jax, "sharded": sharded, "in_names": in_names,
        "out_names": out_names, "out_avals": out_avals,
        "zero_outs": zero_outs, "mesh": mesh,
        "sharding": NamedSharding(mesh, PartitionSpec("core")),
    }
    _NC_CACHE["runner"] = runner
    return runner


def _run_hw(in_maps, bench_iters=0):
    r = _get_runner()
    jax = r["jax"]
    concat_in = [
        np.concatenate([np.asarray(in_maps[c][n]) for c in range(NCORES)],
                       axis=0) for n in r["in_names"]]
    concat_zeros = [np.zeros((NCORES * z.shape[0], *z.shape[1:]), z.dtype)
                    for z in r["zero_outs"]]
    din = [jax.device_put(a, r["sharding"]) for a in concat_in]
    out = r["sharded"](*din, *[jax.device_put(z, r["sharding"])
                               for z in concat_zeros])
    jax.block_until_ready(out)
    out_np = [np.asarray(o) for o in out]
    if bench_iters:
        import time
        import jax.numpy as jnp
        zshapes = [(NCORES * z.shape[0], *z.shape[1:]) for z in r["zero_outs"]]
        zdtypes = [z.dtype for z in r["zero_outs"]]
        zfn = jax.jit(
            lambda: tuple(jnp.zeros(s, d) for s, d in zip(zshapes, zdtypes)),
            out_shardings=tuple(r["sharding"] for _ in zshapes))

        def run_k(k):
            dzs = [zfn() for _ in range(k)]
            jax.block_until_ready(dzs)
            t0 = time.perf_counter()
            outs = [r["sharded"](*din, *dzs[i]) for i in range(k)]
            jax.block_until_ready(outs)
            return time.perf_counter() - t0

        run_k(2)
        k1, k2 = bench_iters, 2 * bench_iters
        t_a = min(run_k(k1) for _ in range(2))
        t_b = min(run_k(k2) for _ in range(2))
        per_exec = (t_b - t_a) / (k2 - k1)
        _NC_CACHE["last_exec_time_ns"] = int(per_exec * 1e9)
        _NC_CACHE["bench_times_ns"] = [int(t_a * 1e9), int(t_b * 1e9)]
    results = []
    for c in range(NCORES):
        results.append({
            name: out_np[i].reshape(NCORES, *r["out_avals"][i].shape)[c]
            for i, name in enumerate(r["out_names"])})
    return results


def kernel(**inputs):
    nc = get_nc()
    in_maps = host_prep(inputs)
    if os.environ.get("KBENCH_SIM"):
        from concourse.bass_interp import MultiCoreSim
        sim = MultiCoreSim(nc, num_cores=NCORES)
        for c, core in enumerate(sim.cores.values()):
            for k, v in in_maps[c].items():
                core.tensor(k)[:] = v
        sim.simulate(check_with_hw=False)
        results = [{"out_c": np.array(core.tensor("out_c"))}
                   for core in sim.cores.values()]
        return assemble(results)
    iters = int(os.environ.get("KBENCH_ITERS", "0"))
    results = _run_hw(in_maps, bench_iters=iters)
    return assemble(results)
